# revision 2
# baseline (speedup 1.0000x reference)
"""Trainium2 Bass kernel for CepstralBlock: causal depthwise conv along D
(K=4, per-channel weights) followed by a 128x128 Linear.

Hybrid engine split (the kernel is DMA-bound at ~47us/core, so compute is
balanced to sit under that floor):
  * d < PE_D  : conv folded into the matmul -- out_d = sum_k x_{d-k} @ W_k
    with W_k = diag(w_k) @ W, 4 PSUM-accumulated matmuls per slice (PE only).
  * d >= PE_D : conv computed on the DVE as 4 per-partition-scalar fused
    multiply-adds (scalar_tensor_tensor, bf16 2x mode), then ONE matmul
    with the plain W.  This cuts PE work ~3x for those slices and keeps
    both PE (~34us) and DVE (~33us) under the DMA floor.
  * PSUM -> SBUF copy + bias ride the otherwise-idle ACT (scalar) engine
    via activation(Identity, bias=b, scale=1).

Layout trick: host pre-transposes x to channel-major [C, B, D, S] so that
C=128 sits on the SBUF partition axis (the matmul contraction axis) and no
on-device transposes are needed.  Output comes back [C_out, B, D, S] and is
un-transposed on host.

Sharding: data-parallel over H (64 -> 8 per core), 8 NeuronCores, no
collectives.  Inputs are converted to bf16 on host (PE bf16 is full-rate;
fp32 matmul is not), PSUM accumulates in f32, output is stored bf16.
"""

import sys
import types

sys.path.insert(0, "/opt/trn_rl_repo")

import numpy as np
import ml_dtypes

# Problem shapes (hardcoded; kernel.py must be self-contained).
B = 4
D = 16
H = 64
WD = 64
C = 128
KS = 4
NCORES = 8
HSH = H // NCORES          # 8 H-rows per core
S = HSH * WD               # 512 spatial positions per (b, d) slice

DCH = 4                    # depth slices per x load chunk / output store chunk
PE_D = 9                   # d < PE_D: 4-matmul PSUM fold; d >= PE_D: DVE conv

# Set by test.py to run with NTFF profiling and stash exec time here.
PROFILE = False
NPROF = 4          # traced runs when PROFILE; min exec_time_ns is reported
LAST_EXEC_NS = None
LAST_EXEC_ALL = None
LAST_RESULT = None

_graph_cache = {}


def _install_ntff_hook():
    """Provide antenv.axon_hooks + register the NTFF profile hook if the
    image's antenv package lacks it (needed for trace=True under axon)."""
    try:
        from antenv import axon_hooks  # noqa: F401
        return
    except ImportError:
        pass
    try:
        import antenv
        from trn_agent_boot.trn_boot import _ntff_profile_via_ctypes
    except ImportError:
        return
    mod = types.ModuleType("antenv.axon_hooks")
    mod._hook = None

    def set_axon_ntff_profile_hook(h):
        mod._hook = h

    def get_axon_ntff_profile_hook():
        return mod._hook

    mod.set_axon_ntff_profile_hook = set_axon_ntff_profile_hook
    mod.get_axon_ntff_profile_hook = get_axon_ntff_profile_hook
    sys.modules["antenv.axon_hooks"] = mod
    antenv.axon_hooks = mod
    mod.set_axon_ntff_profile_hook(
        _ntff_profile_via_ctypes("/opt/axon/libaxon_pjrt.so")
    )


def _build_graph():
    import concourse.mybir as mybir
    from concourse import bacc
    from concourse.tile import TileContext

    nc = bacc.Bacc("TRN2", target_bir_lowering=False, debug=False,
                   num_devices=NCORES)
    xt = nc.declare_dram_parameter("xt", [C, B, D, S], mybir.dt.bfloat16,
                                   isOutput=False)
    # W_0..W_3 (= diag(w_k) @ W) then the plain W:  [C, 5*C]
    wk = nc.declare_dram_parameter("wk", [C, (KS + 1) * C], mybir.dt.bfloat16,
                                   isOutput=False)
    # raw per-channel tap weights for the DVE conv path: [C, KS] f32
    wt = nc.declare_dram_parameter("wt", [C, KS], mybir.dt.float32,
                                   isOutput=False)
    bb = nc.declare_dram_parameter("bb", [C, 1], mybir.dt.float32,
                                   isOutput=False)
    out = nc.declare_dram_parameter("out", [C, B, D, S], mybir.dt.bfloat16,
                                    isOutput=True)

    NCH = D // DCH              # chunks per batch
    mult = mybir.AluOpType.mult
    add = mybir.AluOpType.add
    ident = mybir.ActivationFunctionType.Identity

    with TileContext(nc) as tc:
        with (
            tc.tile_pool(name="consts", bufs=1) as cpool,
            tc.tile_pool(name="xin", bufs=2 * NCH) as xpool,
            tc.tile_pool(name="yconv", bufs=6) as ypool,
            tc.tile_pool(name="ostage", bufs=4) as opool,
            tc.tile_pool(name="ps", bufs=6, space="PSUM") as pspool,
            tc.tile_pool(name="warm", bufs=2, space="PSUM") as wpool,
        ):
            # Consts ride the ACT HWDGE ring so the x loads own the SP ring
            # from the first instruction.
            wk_sb = cpool.tile([C, (KS + 1) * C], mybir.dt.bfloat16)
            nc.scalar.dma_start(out=wk_sb[:], in_=wk[:])
            wt_sb = cpool.tile([C, KS], mybir.dt.float32)
            nc.scalar.dma_start(out=wt_sb[:], in_=wt[:])
            b_sb = cpool.tile([C, 1], mybir.dt.float32)
            nc.scalar.dma_start(out=b_sb[:], in_=bb[:])

            # Keep the PE busy while the first x chunks stream in, so the HAM
            # clock gate flips to 2.4 GHz before the real matmuls start.
            warm_src = cpool.tile([C, S], mybir.dt.bfloat16)
            nc.vector.memset(warm_src[:], 0.0)
            for i in range(8):
                wtile = wpool.tile([C, S], mybir.dt.float32, tag="warm",
                                   name=f"warm_{i}")
                nc.tensor.matmul(wtile[:], warm_src[:, 0:C], warm_src[:],
                                 start=True, stop=True)

            # All x loads up front on the SP ring; the tile pool's buf
            # rotation back-pressures them against compute.
            xch = {}                      # (b, chunk) -> tile [C, DCH*S]
            for b in range(B):
                for i in range(NCH):
                    t = xpool.tile([C, DCH * S], mybir.dt.bfloat16, tag="xh",
                                   name=f"xh_{b}_{i}")
                    nc.sync.dma_start(
                        out=t[:],
                        in_=xt[:, b, i * DCH:(i + 1) * DCH].rearrange(
                            "c d s -> c (d s)"),
                    )
                    xch[(b, i)] = t

            def x_slice(b, d):
                t = xch[(b, d // DCH)]
                j = d % DCH
                return t[:, j * S:(j + 1) * S]

            for b in range(B):
                # DVE conv for the back-half slices first: the DVE fills y
                # tiles while the PE chews through the front-half folds.
                ytiles = {}
                for d in range(PE_D, D):
                    t0 = ypool.tile([C, S], mybir.dt.bfloat16, tag="yc",
                                    name=f"yc_{b}_{d}a")
                    t1 = ypool.tile([C, S], mybir.dt.bfloat16, tag="yc",
                                    name=f"yc_{b}_{d}b")
                    nc.vector.tensor_scalar_mul(
                        t0[:], x_slice(b, d - 3), wt_sb[:, 3:4])
                    nc.vector.scalar_tensor_tensor(
                        t1[:], x_slice(b, d - 2), wt_sb[:, 2:3], t0[:],
                        mult, add)
                    nc.vector.scalar_tensor_tensor(
                        t0[:], x_slice(b, d - 1), wt_sb[:, 1:2], t1[:],
                        mult, add)
                    nc.vector.scalar_tensor_tensor(
                        t1[:], x_slice(b, d), wt_sb[:, 0:1], t0[:],
                        mult, add)
                    ytiles[d] = t1

                obs = []
                for i in range(NCH):
                    t = opool.tile([C, DCH * S], mybir.dt.bfloat16, tag="ob",
                                   name=f"ob_{b}_{i}")
                    obs.append(t)

                def o_slice(d):
                    j = d % DCH
                    return obs[d // DCH][:, j * S:(j + 1) * S]

                pss = {}
                # PE front half: conv folded into 1-4 accumulated matmuls.
                for d in range(0, PE_D):
                    ps = pspool.tile([C, S], mybir.dt.float32, tag="ps",
                                     name=f"ps_{b}_{d}")
                    pss[d] = ps
                    ks = [k for k in range(KS) if d - k >= 0]
                    for k in ks:
                        nc.tensor.matmul(
                            ps[:],
                            wk_sb[:, k * C:(k + 1) * C],
                            x_slice(b, d - k),
                            start=(k == 0),
                            stop=(k == ks[-1]),
                        )
                # PE back half: one matmul per slice on the DVE-conv output.
                for d in range(PE_D, D):
                    ps = pspool.tile([C, S], mybir.dt.float32, tag="ps",
                                     name=f"ps_{b}_{d}")
                    pss[d] = ps
                    nc.tensor.matmul(
                        ps[:], wk_sb[:, KS * C:(KS + 1) * C], ytiles[d][:],
                        start=True, stop=True)

                # ACT drains PSUM -> SBUF bf16 with the bias folded in, then
                # the same engine's HWDGE ring streams each finished chunk out.
                for d in range(D):
                    nc.scalar.activation(o_slice(d), pss[d][:], ident,
                                         bias=b_sb[:, 0:1], scale=1.0)
                    if d % DCH == DCH - 1:
                        i = d // DCH
                        nc.scalar.dma_start(
                            out=out[:, b, i * DCH:(i + 1) * DCH].rearrange(
                                "c d s -> c (d s)"),
                            in_=obs[i][:],
                        )
    nc.compile()
    return nc


def _get_graph():
    if "nc" not in _graph_cache:
        _graph_cache["nc"] = _build_graph()
    return _graph_cache["nc"]


def kernel(x, kernel, W, b):
    global LAST_EXEC_NS, LAST_RESULT
    from concourse.bass_utils import run_bass_kernel_spmd

    nc = _get_graph()

    x = np.asarray(x, np.float32)
    kernel = np.asarray(kernel, np.float32)
    W = np.asarray(W, np.float32)
    b = np.asarray(b, np.float32)

    # Host precompute: fold the depthwise filter into 4 Linear weights and
    # append the plain W for the DVE-conv path.
    w_full = np.tile(kernel, (C // kernel.shape[0], 1))          # [C, KS]
    wk_cat = np.concatenate(
        [w_full[:, k:k + 1] * W for k in range(KS)] + [W], axis=1
    ).astype(ml_dtypes.bfloat16)                                 # [C, 5*C]
    wt_raw = np.ascontiguousarray(w_full, dtype=np.float32)      # [C, KS]
    b_col = b.reshape(C, 1).astype(np.float32)

    # Channel-major transpose + H-shard + bf16.
    xbf = x.astype(ml_dtypes.bfloat16)
    xtr = np.transpose(xbf, (4, 0, 1, 2, 3))                     # [C,B,D,H,W]
    in_maps = []
    for i in range(NCORES):
        shard = np.ascontiguousarray(
            xtr[:, :, :, i * HSH:(i + 1) * HSH, :]
        ).reshape(C, B, D, S)
        in_maps.append({"xt": shard, "wk": wk_cat, "wt": wt_raw,
                        "bb": b_col})

    global LAST_EXEC_ALL
    core_ids = list(range(NCORES))
    res = None
    if PROFILE:
        _install_ntff_hook()
        try:
            # Warm run first: the NEFF compile on a cold cache must not
            # happen inside the NTFF capture window.
            run_bass_kernel_spmd(nc, in_maps, core_ids=core_ids)
            times = []
            for _ in range(max(1, NPROF)):
                res = run_bass_kernel_spmd(nc, in_maps, core_ids=core_ids,
                                           trace=True)
                times.append(res.exec_time_ns)
            LAST_EXEC_ALL = times
        except Exception as e:
            print(f"profile run failed ({type(e).__name__}: {e}); "
                  "falling back to non-traced run", file=sys.stderr)
            res = None
    if res is None:
        res = run_bass_kernel_spmd(nc, in_maps, core_ids=core_ids)
        LAST_EXEC_NS = res.exec_time_ns
    else:
        LAST_EXEC_NS = min(t for t in LAST_EXEC_ALL if t is not None)
    LAST_RESULT = res

    # Gather: shard_i[o, b, d, h*WD + w] -> full[b, d, i*HSH + h, w, o]
    o = np.stack([np.asarray(res.results[i]["out"]) for i in range(NCORES)],
                 axis=0).astype(np.float32)
    o = o.reshape(NCORES, C, B, D, HSH, WD)
    o = np.transpose(o, (2, 3, 0, 4, 5, 1)).reshape(B, D, H, WD, C)
    return np.ascontiguousarray(o)


# revision 4
# speedup vs baseline: 1.2082x; 1.2082x over previous
"""Trainium2 Bass kernel for CepstralBlock: causal depthwise conv along D
(K=4, per-channel weights) followed by a 128x128 Linear.

Hybrid engine split (the kernel is DMA-bound at ~47us/core, so compute is
balanced to sit under that floor):
  * fold slices   : conv folded into the matmul -- out_d = sum_k x_{d-k}@W_k
    with W_k = diag(w_k) @ W, 1-4 PSUM-accumulated matmuls per slice.
  * factored slices: conv computed on the DVE as a batched "tree" over a
    contiguous d-group -- per-tap products via tensor_scalar_mul (fast
    single-src mode), pairwise sums via tensor_tensor ADD (bf16 2x mode) --
    then ONE matmul per slice with the plain W.  (scalar_tensor_tensor has
    no 2x uop on TRN2: measured 677ns per 512-elem op, so it is avoided.)
  * PSUM -> SBUF + bias rides the otherwise-idle ACT engine as ONE
    activation(Identity) per [C, 4*S] PSUM chunk.

For b < 3 the factored slices are d=9..15 (back half); for the last batch
they are d=4..7 so the final store chain after the last x chunk lands is
the short all-PE fold path.

Layout: host pre-transposes x to channel-major [C, B, D, S] so C=128 sits
on the SBUF partition axis (the matmul contraction axis).  Data-parallel
over H (64 -> 8 per core), 8 NeuronCores, no collectives.  bf16 compute,
f32 PSUM accumulation, bf16 output.
"""

import sys
import types

sys.path.insert(0, "/opt/trn_rl_repo")

import numpy as np
import ml_dtypes

# Problem shapes (hardcoded; kernel.py must be self-contained).
B = 4
D = 16
H = 64
WD = 64
C = 128
KS = 4
NCORES = 8
HSH = H // NCORES          # 8 H-rows per core
S = HSH * WD               # 512 spatial positions per (b, d) slice

DCH = 4                    # depth slices per x/psum/out chunk
NCH = D // DCH

# Factored (DVE-conv) slice sets per batch.  Groups must be contiguous runs.
FACT_GROUPS = {
    0: [(9, 12), (12, 16)],
    1: [(9, 12), (12, 16)],
    2: [(9, 12), (12, 16)],
    3: [(4, 8)],           # keep the tail chunks on the all-PE fold path
}

# Set by test.py to run with NTFF profiling and stash exec time here.
PROFILE = False
NPROF = 4          # traced runs when PROFILE; min exec_time_ns is reported
LAST_EXEC_NS = None
LAST_EXEC_ALL = None
LAST_RESULT = None

_graph_cache = {}


def _install_ntff_hook():
    """Provide antenv.axon_hooks + register the NTFF profile hook if the
    image's antenv package lacks it (needed for trace=True under axon)."""
    try:
        from antenv import axon_hooks  # noqa: F401
        return
    except ImportError:
        pass
    try:
        import antenv
        from trn_agent_boot.trn_boot import _ntff_profile_via_ctypes
    except ImportError:
        return
    mod = types.ModuleType("antenv.axon_hooks")
    mod._hook = None

    def set_axon_ntff_profile_hook(h):
        mod._hook = h

    def get_axon_ntff_profile_hook():
        return mod._hook

    mod.set_axon_ntff_profile_hook = set_axon_ntff_profile_hook
    mod.get_axon_ntff_profile_hook = get_axon_ntff_profile_hook
    sys.modules["antenv.axon_hooks"] = mod
    antenv.axon_hooks = mod
    mod.set_axon_ntff_profile_hook(
        _ntff_profile_via_ctypes("/opt/axon/libaxon_pjrt.so")
    )


def _build_graph():
    import concourse.mybir as mybir
    from concourse import bacc
    from concourse.tile import TileContext

    nc = bacc.Bacc("TRN2", target_bir_lowering=False, debug=False,
                   num_devices=NCORES)
    xt = nc.declare_dram_parameter("xt", [C, B, D, S], mybir.dt.bfloat16,
                                   isOutput=False)
    # W_0..W_3 (= diag(w_k) @ W) then the plain W:  [C, 5*C]
    wk = nc.declare_dram_parameter("wk", [C, (KS + 1) * C], mybir.dt.bfloat16,
                                   isOutput=False)
    # raw per-channel tap weights for the DVE conv path: [C, KS] f32
    wt = nc.declare_dram_parameter("wt", [C, KS], mybir.dt.float32,
                                   isOutput=False)
    bb = nc.declare_dram_parameter("bb", [C, 1], mybir.dt.float32,
                                   isOutput=False)
    out = nc.declare_dram_parameter("out", [C, B, D, S], mybir.dt.bfloat16,
                                    isOutput=True)

    mult = mybir.AluOpType.mult
    add = mybir.AluOpType.add
    ident = mybir.ActivationFunctionType.Identity

    fact = {b: set() for b in range(B)}
    for b, groups in FACT_GROUPS.items():
        for g0, g1 in groups:
            fact[b].update(range(g0, g1))

    with TileContext(nc) as tc:
        with (
            tc.tile_pool(name="consts", bufs=1) as cpool,
            tc.tile_pool(name="xin", bufs=2 * NCH) as xpool,
            tc.tile_pool(name="prod", bufs=8) as ppool,
            tc.tile_pool(name="yconv", bufs=4) as ypool,
            tc.tile_pool(name="ostage", bufs=4) as opool,
            tc.tile_pool(name="ps", bufs=2, space="PSUM") as pspool,
        ):
            # Consts ride the ACT HWDGE ring so the x loads own the SP ring
            # from the first instruction.
            wk_sb = cpool.tile([C, (KS + 1) * C], mybir.dt.bfloat16)
            nc.scalar.dma_start(out=wk_sb[:], in_=wk[:])
            wt_sb = cpool.tile([C, KS], mybir.dt.float32)
            nc.scalar.dma_start(out=wt_sb[:], in_=wt[:])
            b_sb = cpool.tile([C, 1], mybir.dt.float32)
            nc.scalar.dma_start(out=b_sb[:], in_=bb[:])

            # Keep the PE busy while the first x chunks stream in, so the HAM
            # clock gate flips to 2.4 GHz before the real matmuls start.
            # PSUM is fully claimed by the 2 [C, DCH*S] chunk tiles, so the
            # warmups write quarters of pool tiles (PE-queue order keeps the
            # reuse safe).
            warm_src = cpool.tile([C, S], mybir.dt.bfloat16)
            nc.vector.memset(warm_src[:], 0.0)
            for i in range(2):
                wtile = pspool.tile([C, DCH * S], mybir.dt.float32, tag="ps",
                                    name=f"warm_{i}")
                for q in range(DCH):
                    nc.tensor.matmul(wtile[:, q * S:(q + 1) * S],
                                     warm_src[:, 0:C], warm_src[:],
                                     start=True, stop=True)

            # All x loads up front on the SP ring; the tile pool's buf
            # rotation back-pressures them against compute.
            xch = {}
            for b in range(B):
                for i in range(NCH):
                    t = xpool.tile([C, DCH * S], mybir.dt.bfloat16, tag="xh",
                                   name=f"xh_{b}_{i}")
                    nc.sync.dma_start(
                        out=t[:],
                        in_=xt[:, b, i * DCH:(i + 1) * DCH].rearrange(
                            "c d s -> c (d s)"),
                    )
                    xch[(b, i)] = t

            def x_view(b, d0, d1):
                """[C, (d1-d0)*S] view; [d0, d1) must live in one chunk."""
                i = d0 // DCH
                assert (d1 - 1) // DCH == i, (d0, d1)
                j = d0 % DCH
                return xch[(b, i)][:, j * S:(j + d1 - d0) * S]

            def conv_group(b, g0, g1):
                """DVE tree conv for slices [g0, g1); returns y tile view."""
                L = g1 - g0
                prods = []
                for k in range(KS):
                    tk = ppool.tile([C, DCH * S], mybir.dt.bfloat16,
                                    tag="pp", name=f"pp_{b}_{g0}_{k}")
                    # product tap k over d in [g0, g1) reads x[d-k]; split
                    # the read range [g0-k, g1-k) at chunk boundaries.
                    a = g0 - k
                    while a < g1 - k:
                        bnd = min(g1 - k, (a // DCH + 1) * DCH)
                        o = (a + k - g0) * S
                        nc.vector.tensor_scalar_mul(
                            tk[:, o:o + (bnd - a) * S],
                            x_view(b, a, bnd),
                            wt_sb[:, k:k + 1])
                        a = bnd
                    prods.append(tk)
                n = L * S
                nc.vector.tensor_tensor(prods[3][:, 0:n], prods[3][:, 0:n],
                                        prods[2][:, 0:n], add)
                nc.vector.tensor_tensor(prods[1][:, 0:n], prods[1][:, 0:n],
                                        prods[0][:, 0:n], add)
                y = ypool.tile([C, DCH * S], mybir.dt.bfloat16, tag="yc",
                               name=f"yc_{b}_{g0}")
                nc.vector.tensor_tensor(y[:, 0:n], prods[3][:, 0:n],
                                        prods[1][:, 0:n], add)
                return y

            for b in range(B):
                # DVE conv first so the vector engine races ahead of the PE.
                ys = {}                       # d -> [C, S] view
                for g0, g1 in FACT_GROUPS.get(b, []):
                    y = conv_group(b, g0, g1)
                    for d in range(g0, g1):
                        ys[d] = y[:, (d - g0) * S:(d - g0 + 1) * S]

                for i in range(NCH):
                    psc = pspool.tile([C, DCH * S], mybir.dt.float32,
                                      tag="ps", name=f"ps_{b}_{i}")
                    for d in range(i * DCH, (i + 1) * DCH):
                        pq = psc[:, (d % DCH) * S:(d % DCH + 1) * S]
                        if d in fact[b]:
                            nc.tensor.matmul(
                                pq, wk_sb[:, KS * C:(KS + 1) * C], ys[d],
                                start=True, stop=True)
                        else:
                            ks = [k for k in range(KS) if d - k >= 0]
                            for k in ks:
                                nc.tensor.matmul(
                                    pq, wk_sb[:, k * C:(k + 1) * C],
                                    x_view(b, d - k, d - k + 1),
                                    start=(k == 0), stop=(k == ks[-1]))
                    ob = opool.tile([C, DCH * S], mybir.dt.bfloat16,
                                    tag="ob", name=f"ob_{b}_{i}")
                    nc.scalar.activation(ob[:], psc[:], ident,
                                         bias=b_sb[:, 0:1], scale=1.0)
                    nc.scalar.dma_start(
                        out=out[:, b, i * DCH:(i + 1) * DCH].rearrange(
                            "c d s -> c (d s)"),
                        in_=ob[:],
                    )
    nc.compile()
    return nc


def _get_graph():
    if "nc" not in _graph_cache:
        _graph_cache["nc"] = _build_graph()
    return _graph_cache["nc"]


def kernel(x, kernel, W, b):
    global LAST_EXEC_NS, LAST_RESULT
    from concourse.bass_utils import run_bass_kernel_spmd

    nc = _get_graph()

    x = np.asarray(x, np.float32)
    kernel = np.asarray(kernel, np.float32)
    W = np.asarray(W, np.float32)
    b = np.asarray(b, np.float32)

    # Host precompute: fold the depthwise filter into 4 Linear weights and
    # append the plain W for the DVE-conv path.
    w_full = np.tile(kernel, (C // kernel.shape[0], 1))          # [C, KS]
    wk_cat = np.concatenate(
        [w_full[:, k:k + 1] * W for k in range(KS)] + [W], axis=1
    ).astype(ml_dtypes.bfloat16)                                 # [C, 5*C]
    wt_raw = np.ascontiguousarray(w_full, dtype=np.float32)      # [C, KS]
    b_col = b.reshape(C, 1).astype(np.float32)

    # Channel-major transpose + H-shard + bf16.
    xbf = x.astype(ml_dtypes.bfloat16)
    xtr = np.transpose(xbf, (4, 0, 1, 2, 3))                     # [C,B,D,H,W]
    in_maps = []
    for i in range(NCORES):
        shard = np.ascontiguousarray(
            xtr[:, :, :, i * HSH:(i + 1) * HSH, :]
        ).reshape(C, B, D, S)
        in_maps.append({"xt": shard, "wk": wk_cat, "wt": wt_raw,
                        "bb": b_col})

    global LAST_EXEC_ALL
    core_ids = list(range(NCORES))
    res = None
    if PROFILE:
        _install_ntff_hook()
        try:
            # Warm run first: the NEFF compile on a cold cache must not
            # happen inside the NTFF capture window.
            run_bass_kernel_spmd(nc, in_maps, core_ids=core_ids)
            times = []
            for _ in range(max(1, NPROF)):
                res = run_bass_kernel_spmd(nc, in_maps, core_ids=core_ids,
                                           trace=True)
                times.append(res.exec_time_ns)
            LAST_EXEC_ALL = times
        except Exception as e:
            print(f"profile run failed ({type(e).__name__}: {e}); "
                  "falling back to non-traced run", file=sys.stderr)
            res = None
    if res is None:
        res = run_bass_kernel_spmd(nc, in_maps, core_ids=core_ids)
        LAST_EXEC_NS = res.exec_time_ns
    else:
        LAST_EXEC_NS = min(t for t in LAST_EXEC_ALL if t is not None)
    LAST_RESULT = res

    # Gather: shard_i[o, b, d, h*WD + w] -> full[b, d, i*HSH + h, w, o]
    o = np.stack([np.asarray(res.results[i]["out"]) for i in range(NCORES)],
                 axis=0).astype(np.float32)
    o = o.reshape(NCORES, C, B, D, HSH, WD)
    o = np.transpose(o, (2, 3, 0, 4, 5, 1)).reshape(B, D, H, WD, C)
    return np.ascontiguousarray(o)
